# revision 1
# baseline (speedup 1.0000x reference)
"""MeshPool kernel for Trainium2 (8 NeuronCores, SPMD).

pooled = (relationships / rowsum(relationships)) @ features

Sharding: relationships row-blocks across 8 cores, features replicated.
Per core: R_local [1500, 24000] f32, F [24000, 32] f32 -> out [1500, 32].

Device algorithm (per core):
  - Host passes F augmented with a ones column, padded to 34 columns and
    to a multiple of 128 rows -> F_aug [n_kc*128, 34]; the GEMM against
    F_aug also produces row sums (column 32 of the product).
  - R streams in natural layout [m(128part), k] via large DMAs; each
    [128m x 128k] chunk is transposed on the TensorEngine (transpose-mode
    matmul vs identity) into PSUM, copied to SBUF (alternating DVE/ACT),
    and used as the moving operand of an accumulating matmul with the
    F_aug chunk stationary:  accT[34, m_tile] += F_aug[k]^T @ R^T[k, mt].
  - Epilogue: accT transposed back via a REGULAR f32r matmul against the
    identity (transpose-mode has ISA restrictions for 4-byte dtypes that
    odd 34-row tiles violate); out = acc[:, :32] * (1/acc[:, 32]).
  - Matmul-feeding tensors are float32r (fp32 bits; reduced-precision
    multiply, fp32 PSUM accumulate) for 4x PE throughput vs fp32.
  - TRN2 instructions carry at most ONE semaphore wait; a post-pass
    splits extra waits onto preceding NoOps on the same engine queue.
"""

import numpy as np
from contextlib import ExitStack

import concourse.bass as bass
import concourse.mybir as mybir
import concourse.tile as tile
from concourse.bass_utils import run_bass_kernel_spmd

N_CORES = 8
M_TOTAL = 12000
K_DIM = 24000
F_DIM = 32

P = 128
F32 = mybir.dt.float32
F32R = mybir.dt.float32r


def _cdiv(a, b):
    return -(-a // b)


def _split_multi_waits(nc):
    """TRN2 ISA: one sem-wait slot per instruction. Move extras to NoOps."""
    for fn in nc.m.functions:
        for bb in fn.blocks:
            new = []
            for ins in bb.instructions:
                si = ins.sync_info
                if si is not None and len(si.on_wait) > 1:
                    for w in si.on_wait[:-1]:
                        new.append(
                            mybir.InstNoOp(
                                name=nc.get_next_instruction_name(),
                                engine=ins.engine,
                                ins=[],
                                outs=[],
                                sync_info=mybir.SyncInfo(on_wait=[w], on_update=[]),
                            )
                        )
                    ins.sync_info = mybir.SyncInfo(
                        on_wait=[si.on_wait[-1]], on_update=si.on_update
                    )
                new.append(ins)
            bb.instructions = new
    return nc


def build_nc(
    m_local=M_TOTAL // N_CORES,
    k_dim=K_DIM,
    f_dim=F_DIM,
    m_tile=512,
    k_seg=4096,
    use_f32r=True,
    split_waits=True,
    stage="full",
):
    nc = bass.Bass()
    DT = F32R if use_f32r else F32
    fa = f_dim + 2  # +1 ones column (row sums), +1 zero pad to keep fa even
    n_kc = _cdiv(k_dim, P)  # k chunks of 128 (k zero-padded to full chunks)
    k_pad = n_kc * P
    assert k_seg % P == 0 and m_tile % P == 0

    rel = nc.declare_dram_parameter(
        "relationships", [m_local, k_dim], DT, isOutput=False
    )
    feat = nc.declare_dram_parameter("features_aug", [k_pad, fa], DT, isOutput=False)
    identd = nc.declare_dram_parameter("ident", [P, P], DT, isOutput=False)
    out = nc.declare_dram_parameter("out", [m_local, f_dim], F32, isOutput=True)

    with tile.TileContext(nc) as tc, ExitStack() as ctx:
        singles = ctx.enter_context(tc.tile_pool(name="singles", bufs=1))
        nat_pool = ctx.enter_context(tc.tile_pool(name="nat", bufs=2))
        rt_pool = ctx.enter_context(tc.tile_pool(name="rt", bufs=4))
        acc_sb_pool = ctx.enter_context(tc.tile_pool(name="accsb", bufs=2))
        out_pool = ctx.enter_context(tc.tile_pool(name="outp", bufs=4))
        tp_psum = ctx.enter_context(tc.tile_pool(name="tp", bufs=3, space="PSUM"))
        acc_psum = ctx.enter_context(tc.tile_pool(name="acc", bufs=3, space="PSUM"))
        scr_psum = ctx.enter_context(tc.tile_pool(name="scr", bufs=2, space="PSUM"))

        ident = singles.tile([P, P], DT)
        nc.sync.dma_start(out=ident, in_=identd[:, :])

        # F_aug chunks: f_sb[p, c, j] = F_aug[c*128+p, j]
        f_sb = singles.tile([P, n_kc, fa], DT)
        nc.sync.dma_start(
            out=f_sb, in_=feat[:, :].rearrange("(c p) j -> p c j", p=P)
        )

        # Warmup PE ops (regular f32r matmuls -> no transpose-mode ISA
        # restrictions): absorb the ident / f_sb DMA waits so later PE
        # instructions never need a second wait slot.
        scr = scr_psum.tile([P, P], F32, tag="scr")
        nc.tensor.matmul(scr[:P, :P], ident, ident)
        scr = scr_psum.tile([P, P], F32, tag="scr")
        nc.tensor.matmul(scr[:fa, :P], f_sb[:, 0, :], ident)

        n_mt = _cdiv(m_local, m_tile)
        n_seg = _cdiv(k_pad, k_seg) if stage != "null" else 0
        if stage == "null":
            for i in range(_cdiv(m_local, P)):
                sub_w = min(P, m_local - i * P)
                nc.sync.dma_start(
                    out=out[i * P : i * P + sub_w, :],
                    in_=ident[:sub_w, :f_dim].bitcast(F32),
                )
        for mt in range(n_mt if stage != "null" else 0):
            m0 = mt * m_tile
            m_w = min(m_tile, m_local - m0)
            n_sub = _cdiv(m_w, P)
            acc = acc_psum.tile([fa, m_tile], F32, tag="acc")
            kc_global = 0
            for s in range(n_seg):
                k0 = s * k_seg
                k_w = min(k_seg, k_pad - k0)
                k_real = min(k_seg, k_dim - k0)  # columns actually in DRAM
                nat = nat_pool.tile([P, n_sub, k_seg], DT, tag="nat")
                if m_w % P == 0:
                    nc.sync.dma_start(
                        out=nat[:, :, :k_real],
                        in_=rel[m0 : m0 + m_w, k0 : k0 + k_real].rearrange(
                            "(i p) j -> p i j", p=P
                        ),
                    )
                else:
                    for i in range(n_sub):
                        sub_w = min(P, m_w - i * P)
                        nc.sync.dma_start(
                            out=nat[:sub_w, i, :k_real],
                            in_=rel[
                                m0 + i * P : m0 + i * P + sub_w, k0 : k0 + k_real
                            ],
                        )
                # stray weight load reading the fresh nat tile: soaks up the
                # DMA wait on PE without writing PSUM (no WAW side effects);
                # the next real matmul/transpose reloads weights anyway.
                nc.tensor.ldweights(nat[0:1, 0, 0:32].bitcast(mybir.dt.bfloat16))
                # Columns k_real:k_w (last segment only) are left as stale
                # SBUF data — always finite (prior R values or zeros) — and
                # meet only the zero rows of padded F_aug, contributing 0.
                for c in range(k_w // P if stage != "dma" else 0):
                    tp = tp_psum.tile([P, m_tile], DT, tag="tp")
                    for i in range(n_sub):
                        sub_w = min(P, m_w - i * P)
                        nc.tensor.transpose(
                            tp[:P, i * P : i * P + sub_w],
                            nat[:sub_w, i, c * P : (c + 1) * P],
                            ident[:sub_w, :sub_w],
                        )
                    rt = rt_pool.tile([P, m_tile], DT, tag="rt")
                    if stage != "nocopy":
                        cp_eng = nc.vector if (kc_global % 2 == 0) else nc.scalar
                        if cp_eng is nc.vector:
                            cp_eng.tensor_copy(rt[:P, :m_w], tp[:P, :m_w])
                        else:
                            cp_eng.copy(rt[:P, :m_w], tp[:P, :m_w])
                    if stage == "full":
                        nc.tensor.matmul(
                            acc[:, :m_w],
                            f_sb[:, kc_global, :],
                            rt[:, :m_w],
                            start=(kc_global == 0),
                            stop=(kc_global == n_kc - 1),
                        )
                    kc_global += 1
            if stage != "full":  # timing-only: fabricate the output cheaply
                for i in range(n_sub):
                    sub_w = min(P, m_w - i * P)
                    nc.sync.dma_start(
                        out=out[m0 + i * P : m0 + i * P + sub_w, :],
                        in_=ident[:sub_w, :f_dim].bitcast(F32),
                    )
                continue
            # epilogue: transpose back (regular matmul), divide by row sums
            acc_sb = acc_sb_pool.tile([fa, m_tile], DT, tag="accsb")
            nc.vector.tensor_copy(acc_sb[:, :m_w], acc[:, :m_w])
            for i in range(n_sub):
                sub_w = min(P, m_w - i * P)
                tpo = scr_psum.tile([P, P], F32, tag="scr")
                nc.tensor.matmul(
                    tpo[:sub_w, :fa],
                    acc_sb[:, i * P : i * P + sub_w],
                    ident[:fa, :fa],
                )
                rs = out_pool.tile([P, 1], F32, tag="rs")
                nc.vector.reciprocal(rs[:sub_w], tpo[:sub_w, f_dim : f_dim + 1])
                ot = out_pool.tile([P, f_dim], F32, tag="ot")
                nc.vector.tensor_scalar_mul(ot[:sub_w], tpo[:sub_w, :f_dim], rs[:sub_w])
                nc.sync.dma_start(
                    out=out[m0 + i * P : m0 + i * P + sub_w, :], in_=ot[:sub_w]
                )
    return _split_multi_waits(nc) if split_waits else nc


_NC_CACHE = {}


def _get_nc(key):
    if key not in _NC_CACHE:
        _NC_CACHE[key] = build_nc(*key)
    return _NC_CACHE[key]


def make_aug_inputs(features, relationships, n_cores=N_CORES):
    """Host-side prep: shard R row-wise; augment/pad F; identity matrix."""
    m_total, k_dim = relationships.shape
    _, f_dim = features.shape
    m_local = m_total // n_cores
    n_kc = _cdiv(k_dim, P)
    f_aug = np.zeros((n_kc * P, f_dim + 2), dtype=np.float32)
    f_aug[:k_dim, :f_dim] = features
    f_aug[:k_dim, f_dim] = 1.0
    ident = np.eye(P, dtype=np.float32)
    in_maps = [
        {
            "relationships": np.ascontiguousarray(
                relationships[c * m_local : (c + 1) * m_local]
            ),
            "features_aug": f_aug,
            "ident": ident,
        }
        for c in range(n_cores)
    ]
    return in_maps, m_local


def kernel(features: np.ndarray, relationships: np.ndarray) -> np.ndarray:
    features = np.asarray(features, dtype=np.float32)
    relationships = np.asarray(relationships, dtype=np.float32)
    m_total, k_dim = relationships.shape
    k2, f_dim = features.shape
    assert k2 == k_dim
    assert m_total % N_CORES == 0
    m_local = m_total // N_CORES

    nc = _get_nc((m_local, k_dim, f_dim))
    in_maps, _ = make_aug_inputs(features, relationships)
    last_exc = None
    for _attempt in range(3):  # transient NRT device faults: retry
        try:
            res = run_bass_kernel_spmd(nc, in_maps, core_ids=list(range(N_CORES)))
            break
        except Exception as exc:  # noqa: BLE001
            last_exc = exc
    else:
        raise last_exc
    return np.concatenate([res.results[c]["out"] for c in range(N_CORES)], axis=0)


if __name__ == "__main__":
    rng = np.random.default_rng(0)
    m, k, f = 24, 48, 32  # tiny local smoke (shapes must divide by cores)
    feats = rng.standard_normal((k, f), dtype=np.float32)
    rels = rng.random((N_CORES * m, k), dtype=np.float32)
    got = kernel(feats, rels)
    want = (rels / rels.sum(1, keepdims=True)) @ feats
    err = np.abs(got - want).max() / np.abs(want).max()
    print("rel err:", err)

